# revision 27
# baseline (speedup 1.0000x reference)
"""Chamfer-KL loss kernel for Trainium2 (8 NeuronCores, batch-parallel).

Packed value+index argmin: inputs are quantized to a 2^-3 grid, so the
augmented bf16 matmul

  P[i,j] = 64 - dist(i,j) + (4095-j)*2^-18

is EXACT in fp32 PSUM (all terms are multiples of 2^-18, partial sums stay
under 2^24 ulps, iota rows accumulate last in K-order). A single DVE
reduce_max over PSUM then yields both the max value and, in its low mantissa
bits, the argmin column: j = 4095 - (u32(v * 2^18) & 0xFFF). The reversed
iota makes ties break toward the smallest j, matching jnp.argmin.

This removes the fp16 stage, both ACT copies, MAX8 and FIND_INDEX8 of the
classic design (FIND_INDEX8/MAX8 run at 1x: 4.4us each per tile -- the old
kernels were entirely DVE-bound on them). DVE work per tile drops from
~8.8us to ~4.5us (two 2048-wide 1x reduce_max from PSUM -- the floor, since
only the DVE can compare and it must visit every matrix element once).

Per 8-tile block the indices are extracted (4 tiny uint ops), 8 indirect
row-gathers fire, and the exact fp32 KL for that block runs spread across
GPSIMD (subs/mults), ACT (exp/square) and DVE (reduce), overlapping the
main loop. The two layouts (argmin over gts / over preds) interleave
tile-by-tile to keep two independent dependency chains in flight.
"""

import numpy as np

BS, N, D = 8, 4096, 32
NT = N // 128  # 32 partition tiles
KP = D + 6     # 32 features + 2 own-norm rows + 2 other-norm rows + 2 iota rows

_NC_CACHE = {}


def _build():
    from contextlib import ExitStack

    import concourse.mybir as mybir
    from concourse import bacc
    from concourse.bass import IndirectOffsetOnAxis
    from concourse.tile import TileContext

    f32 = mybir.dt.float32
    bf16 = mybir.dt.bfloat16
    u32 = mybir.dt.uint32
    AF = mybir.ActivationFunctionType
    ALU = mybir.AluOpType

    nc = bacc.Bacc(None, target_bir_lowering=False)
    xS = nc.dram_tensor("xS", [KP, N], bf16, kind="ExternalInput")
    xM = nc.dram_tensor("xM", [KP, N], bf16, kind="ExternalInput")
    yS = nc.dram_tensor("yS", [KP, N], bf16, kind="ExternalInput")
    yM = nc.dram_tensor("yM", [KP, N], bf16, kind="ExternalInput")
    cat_p = nc.dram_tensor("cat_p", [N, 2 * D], f32, kind="ExternalInput")
    cat_g = nc.dram_tensor("cat_g", [N, 2 * D], f32, kind="ExternalInput")
    loss = nc.dram_tensor("loss", [1, 1], f32, kind="ExternalOutput")

    with TileContext(nc) as tc:
        with ExitStack() as ctx:
            const = ctx.enter_context(tc.tile_pool(name="const", bufs=1))
            klp = ctx.enter_context(tc.tile_pool(name="klp", bufs=6))
            exp = ctx.enter_context(tc.tile_pool(name="exp", bufs=4))
            psum_pool = ctx.enter_context(
                tc.tile_pool(name="psum", bufs=2, space="PSUM")
            )

            xS_sb = const.tile([KP, N], bf16, tag="xS_sb")
            xM_sb = const.tile([KP, N], bf16, tag="xM_sb")
            yS_sb = const.tile([KP, N], bf16, tag="yS_sb")
            yM_sb = const.tile([KP, N], bf16, tag="yM_sb")
            nat_p = const.tile([128, NT, 2 * D], f32, tag="nat_p")
            nat_g = const.tile([128, NT, 2 * D], f32, tag="nat_g")
            gath_g = const.tile([128, NT, 2 * D], f32, tag="gath_g")
            gath_p = const.tile([128, NT, 2 * D], f32, tag="gath_p")
            klacc = const.tile([128, NT], f32, tag="klacc")
            vcol_y = const.tile([128, 2 * NT], f32, tag="vcol_y")
            vcol_x = const.tile([128, 2 * NT], f32, tag="vcol_x")
            args_y = const.tile([128, NT], u32, tag="args_y")
            args_x = const.tile([128, NT], u32, tag="args_x")

            # spread the operand loads across both HWDGE engines' rings
            # (sync + scalar) so all four land in parallel
            nc.sync.dma_start(out=xS_sb[:, :], in_=xS[:, :])
            nc.scalar.dma_start(out=yM_sb[:, :], in_=yM[:, :])
            nc.sync.dma_start(out=yS_sb[:, :], in_=yS[:, :])
            nc.scalar.dma_start(out=xM_sb[:, :], in_=xM[:, :])

            KB = 8  # tiles per extraction/gather/KL block

            KG = 4  # tiles per extract+gather burst (finer => shorter tail)

            def extract_block(vcol, args, g):
                """args[:, c0:c1] = 4095 - (u32(max(vh0,vh1) * 2^18) & 0xFFF)"""
                c0, c1 = g * KG, (g + 1) * KG
                vt = klp.tile([128, KG], f32, tag="ex_vt")
                nc.vector.tensor_tensor(
                    vt[:, :],
                    vcol[:, 2 * c0 : 2 * c1 : 2],
                    vcol[:, 2 * c0 + 1 : 2 * c1 : 2],
                    op=ALU.max,
                )
                nc.vector.tensor_scalar_mul(vt[:, :], vt[:, :], float(2**18))
                ui = klp.tile([128, KG], u32, tag="ex_ui")
                nc.vector.tensor_copy(ui[:, :], vt[:, :])
                nc.vector.tensor_scalar(
                    args[:, c0:c1], ui[:, :], 4095, 4095,
                    op0=ALU.bitwise_and, op1=ALU.bitwise_xor,
                )

            def kl_block(gathered, first, b):
                c0, c1 = b * KB, (b + 1) * KB
                if first:
                    # loss_2 side: p = natural preds, o = gathered gts
                    mu_pv = nat_p[:, c0:c1, 0:D]
                    lv_pv = nat_p[:, c0:c1, D : 2 * D]
                    mu_ov = gathered[:, c0:c1, 0:D]
                    lv_ov = gathered[:, c0:c1, D : 2 * D]
                else:
                    # loss_1 side: p = gathered preds, o = natural gts
                    mu_pv = gathered[:, c0:c1, 0:D]
                    lv_pv = gathered[:, c0:c1, D : 2 * D]
                    mu_ov = nat_g[:, c0:c1, 0:D]
                    lv_ov = nat_g[:, c0:c1, D : 2 * D]
                # S = sum_d (t1 - exp(t1) - (mu_p-mu_o)^2 * exp(-lv_o)),
                # t1 = lv_p - lv_o.  (the "+1" per dim is folded in later)
                t1 = klp.tile([128, KB, D], f32, tag="kl_t1")
                e1 = klp.tile([128, KB, D], f32, tag="kl_e1")
                dm = klp.tile([128, KB, D], f32, tag="kl_dm")
                en = klp.tile([128, KB, D], f32, tag="kl_en")
                nc.gpsimd.tensor_sub(t1[:, :, :], lv_pv, lv_ov)
                nc.scalar.activation(e1[:, :, :], t1[:, :, :], AF.Exp)
                nc.gpsimd.tensor_sub(dm[:, :, :], mu_pv, mu_ov)
                nc.scalar.activation(dm[:, :, :], dm[:, :, :], AF.Square)
                nc.scalar.activation(en[:, :, :], lv_ov, AF.Exp, scale=-1.0)
                nc.gpsimd.tensor_sub(t1[:, :, :], t1[:, :, :], e1[:, :, :])
                nc.gpsimd.tensor_mul(dm[:, :, :], dm[:, :, :], en[:, :, :])
                nc.gpsimd.tensor_sub(t1[:, :, :], t1[:, :, :], dm[:, :, :])
                if first:
                    nc.vector.reduce_sum(
                        klacc[:, c0:c1], t1[:, :, :], axis=mybir.AxisListType.X
                    )
                else:
                    red = klp.tile([128, KB], f32, tag="kl_red")
                    nc.vector.reduce_sum(
                        red[:, :], t1[:, :, :], axis=mybir.AxisListType.X
                    )
                    nc.gpsimd.tensor_add(
                        klacc[:, c0:c1], klacc[:, c0:c1], red[:, :]
                    )

            # --- main scans: two interleaved independent chains ---
            passes = (
                (xS_sb, yM_sb, vcol_y, args_y, gath_g, cat_g),
                (yS_sb, xM_sb, vcol_x, args_x, gath_p, cat_p),
            )
            for t in range(NT):
                if t == 1:
                    # the (mu|logvar) tables are first read by kl_block at
                    # t=15; loading them here keeps the DMA queues clear for
                    # the matmul operands that gate the first tiles.
                    nc.sync.dma_start(
                        out=nat_p[:, :, :],
                        in_=cat_p.rearrange("(t p) c -> p t c", p=128),
                    )
                    nc.scalar.dma_start(
                        out=nat_g[:, :, :],
                        in_=cat_g.rearrange("(t p) c -> p t c", p=128),
                    )
                for stat, mov, vcol, args, gath, cat in passes:
                    for h in range(2):
                        ps = psum_pool.tile([128, 2048], f32, tag="ps")
                        for q in range(4):
                            c = h * 4 + q
                            nc.tensor.matmul(
                                ps[:, q * 512 : (q + 1) * 512],
                                lhsT=stat[:, t * 128 : (t + 1) * 128],
                                rhs=mov[:, c * 512 : (c + 1) * 512],
                                start=True,
                                stop=True,
                            )
                        nc.vector.reduce_max(
                            vcol[:, 2 * t + h : 2 * t + h + 1],
                            ps[:, :],
                            axis=mybir.AxisListType.X,
                        )
                if t % KB == KB - 1:
                    # KL for the PREVIOUS block: its gathers completed ~8
                    # tiles ago, so the chain never stalls any queue, and it
                    # sits ahead of this block's gather burst on GPSIMD.
                    b = t // KB
                    if b > 0:
                        kl_block(gath_g, first=True, b=b - 1)
                        kl_block(gath_p, first=False, b=b - 1)
                if t % KG == KG - 1:
                    g = t // KG
                    for stat, mov, vcol, args, gath, cat in passes:
                        extract_block(vcol, args, g)
                        for tt in range(g * KG, (g + 1) * KG):
                            nc.gpsimd.indirect_dma_start(
                                gath[:, tt, :],
                                None,
                                cat[:, :],
                                IndirectOffsetOnAxis(
                                    ap=args[:, tt : tt + 1], axis=0
                                ),
                            )
            kl_block(gath_g, first=True, b=NT // KB - 1)
            kl_block(gath_p, first=False, b=NT // KB - 1)

            # fold the two "+ sum_d 1 = +D" constants (one per side)
            nc.vector.tensor_scalar_add(klacc[:, :], klacc[:, :], float(2 * D))

            # partition-sum via ones-vector matmul (exact fp32 in PSUM)
            ones_col = const.tile([128, 1], f32, tag="ones_col")
            nc.vector.memset(ones_col[:, :], 1.0)
            ps_fin = psum_pool.tile([128, 2048], f32, tag="ps")
            nc.tensor.matmul(
                ps_fin[0:1, 0:NT],
                lhsT=ones_col[:, :],
                rhs=klacc[:, :],
                start=True,
                stop=True,
            )
            fin = exp.tile([1, 1], f32, tag="fin")
            nc.vector.reduce_sum(
                fin[:, :], ps_fin[0:1, 0:NT], axis=mybir.AxisListType.X
            )
            # loss = 0.5*(l1+l2), each l = -0.5*S  ->  -0.25*(S1+S2)
            nc.vector.tensor_scalar_mul(fin[:, :], fin[:, :], -0.25)
            nc.sync.dma_start(out=loss[:, :], in_=fin[:, :])

    nc.finalize()
    return nc


def _get_nc():
    if "nc" not in _NC_CACHE:
        _NC_CACHE["nc"] = _build()
    return _NC_CACHE["nc"]


def _pack_operands(q, other_q):
    """Rows for one layout, this side's points `q` [N, D] on the 2^-3 grid.

    As stationary (KP rows): [2*q.T ; -|q|^2 hi ; lo ; 1 ; 1 ; 1 ; 1]
    As moving:               [q.T   ; 1 ; 1 ; (64-|q|^2) hi ; lo ; ihi ; ilo]
    """
    import ml_dtypes

    bf16 = ml_dtypes.bfloat16
    n2 = (q * q).sum(-1)  # |q|^2, multiples of 2^-6
    ones = np.ones((N,), np.float64)

    ns = -n2
    ns_hi = np.floor(ns)
    ns_lo = ns - ns_hi
    stat = np.concatenate(
        [2.0 * q.T, ns_hi[None], ns_lo[None], ones[None], ones[None],
         ones[None], ones[None]], 0
    )

    nm = 64.0 - n2
    nm_hi = np.floor(nm)
    nm_lo = nm - nm_hi
    j = 4095 - np.arange(N)
    ihi = (j >> 6).astype(np.float64) * 2.0**-12
    ilo = (j & 63).astype(np.float64) * 2.0**-18
    mov = np.concatenate(
        [q.T, ones[None], ones[None], nm_hi[None], nm_lo[None],
         ihi[None], ilo[None]], 0
    )
    return (
        np.ascontiguousarray(stat).astype(bf16),
        np.ascontiguousarray(mov).astype(bf16),
    )


def _host_prep(mu_p, lv_p, mu_g, lv_g):
    xq = np.clip(np.round(mu_p.astype(np.float64) * 8) / 8, -8, 8)
    yq = np.clip(np.round(mu_g.astype(np.float64) * 8) / 8, -8, 8)
    xS, xM = _pack_operands(xq, yq)
    yS, yM = _pack_operands(yq, xq)
    cat_p = np.ascontiguousarray(
        np.concatenate([mu_p, lv_p], 1).astype(np.float32)
    )
    cat_g = np.ascontiguousarray(
        np.concatenate([mu_g, lv_g], 1).astype(np.float32)
    )
    return {
        "xS": xS, "xM": xM, "yS": yS, "yM": yM,
        "cat_p": cat_p, "cat_g": cat_g,
    }


def make_in_maps(mu_preds, logvar_preds, mu_gts, logvar_gts):
    mu_preds = np.asarray(mu_preds, dtype=np.float32)
    logvar_preds = np.asarray(logvar_preds, dtype=np.float32)
    mu_gts = np.asarray(mu_gts, dtype=np.float32)
    logvar_gts = np.asarray(logvar_gts, dtype=np.float32)
    return [
        _host_prep(mu_preds[b], logvar_preds[b], mu_gts[b], logvar_gts[b])
        for b in range(BS)
    ]


def run(in_maps, trace=False):
    from concourse.bass_utils import run_bass_kernel_spmd

    nc = _get_nc()
    res = run_bass_kernel_spmd(nc, in_maps, list(range(BS)), trace=trace)
    out = np.array(
        [np.asarray(res.results[b]["loss"]).reshape(()) for b in range(BS)],
        dtype=np.float32,
    )
    return out, res


def kernel(mu_preds, logvar_preds, mu_gts, logvar_gts):
    in_maps = make_in_maps(mu_preds, logvar_preds, mu_gts, logvar_gts)
    out, _ = run(in_maps)
    return out
